# revision 26
# baseline (speedup 1.0000x reference)
"""CrossModalAttentionScorer Trainium2 kernel, v9 (Bass/Tile, 8 NeuronCores).

Same math as v3 (G-trick for scores, attn@Y for the combine's attended
block) plus two structural changes:

  1. fp16 everywhere on the score/combine path (11-bit mantissa ~= f32r
     precision for these ~N(0,1) values, half the DMA bytes); bf16 only for
     the unnormalized softmax path (exp(s-60) spans up to e^30, beyond fp16
     range). All matmuls are same-dtype (the BIR verifier forbids mixing
     f32/f32r with 16-bit operands).

  2. Both 512-row chunks of a batch are processed together, packed along
     the 128-partition axis (chunk 0 in partitions 0..63, chunk 1 in
     64..127). Every T=64-tall stage then runs as two concurrent 64-wide
     PE-array tiles via tile_position (auto-derived from base partitions):
       - scores: 2 col-groups (one per chunk, shared G weights)
       - attended / attn@Y: 2 row-groups (disjoint contraction halves)
       - exp / Z / 1/Z / normalize: single full-128-partition instructions
     This halves the PE time of every half-width stage; on HW the paired
     matmuls overlap almost perfectly (the cost model charges them serially,
     so TimelineSim over-reports this kernel).

Per-batch PE rows ~= 48.2k span-cycles; ~80us/core theoretical at 2.4GHz.
The combine matmuls stay deferred (fillers) and are woven into the next
batch's q-phase / attention-phase latency gaps; leftovers drain only after
the LAST rep so they cover the next rep's boundary windows. Output DMAs go
per-j on the final batch so the tail pipelines, and the startup DMA FIFO is
ordered strictly by first use (contiguous loads merged, tiny constants
deferred behind the first anchor chunks).
"""
import numpy as np

import concourse.bacc as bacc
import concourse.tile as tile
import concourse.mybir as mybir
from concourse.bass_utils import run_bass_kernel_spmd
from concourse.masks import make_identity

B, A, T, D, H = 32, 1024, 64, 512, 512
NCORES = 8
PB = B // NCORES          # batches per core = 4
P = 128                   # partitions
DT = D // P               # 4 d-tiles
HT = H // P               # 4 h-tiles
CT = 3 * H // P           # 12 c-tiles (concat dim)
ACH = 512                 # a-chunk (moving-dim) size
NCH = A // ACH            # 2 chunks per batch (packed along partitions)
AT_CH = ACH // P          # 4 a-tiles per chunk
SHIFT = 60.0              # fixed softmax shift (replaces row max)

F32 = mybir.dt.float32
F32R = mybir.dt.float32r
AFT = mybir.ActivationFunctionType

# FP16 mode: score/combine path in fp16 (half DMA bytes, ~f32r precision),
# softmax unnormalized path in bf16 (range). FP16=False reverts every matmul
# operand to f32r (v3 dtypes) while keeping the v5 packed structure.
FP16 = True
F16 = mybir.dt.float16 if FP16 else mybir.dt.float32r
BF16 = mybir.dt.bfloat16 if FP16 else mybir.dt.float32r

_CACHE = {}


def build(reps: int = 1):
    """Build the per-core Bass module (4 batches of the problem).

    reps>1 repeats the whole computation in one NEFF for slope timing."""
    nc = bacc.Bacc(None, target_bir_lowering=False, debug=False)

    aT = nc.dram_tensor("aT", [PB, NCH, P, DT * ACH], F16, kind="ExternalInput")
    boot = nc.dram_tensor("boot", [P, T + H], F16, kind="ExternalInput")
    qT = nc.dram_tensor("qT", [P, DT * PB * T], F16, kind="ExternalInput")
    mb2 = nc.dram_tensor("mb2", [2 * T, PB], F32, kind="ExternalInput")
    wq = nc.dram_tensor("wq", [P, DT * H], F16, kind="ExternalInput")
    wrT = nc.dram_tensor("wrT", [P, HT * D], F16, kind="ExternalInput")
    wc = nc.dram_tensor("wc", [P, CT * H], F16, kind="ExternalInput")
    bc = nc.dram_tensor("bc", [1, H], F16, kind="ExternalInput")
    x = nc.dram_tensor("x", [PB, NCH, P, AT_CH * H], mybir.dt.bfloat16, kind="ExternalOutput")

    with tile.TileContext(nc) as tc:
        with (
            tc.tile_pool(name="const", bufs=1) as const,
            tc.tile_pool(name="perb", bufs=3) as perb,
            tc.tile_pool(name="chunk", bufs=2) as chunk,
            tc.tile_pool(name="small", bufs=4) as small,
            tc.tile_pool(name="stage", bufs=4) as stage,
            tc.tile_pool(name="psum", bufs=4, space="PSUM") as psum,
        ):
            # ---- constants (load order = need order) ----
            boot_sb = const.tile([P, T + H], F16, name="boot_sb")
            nc.sync.dma_start(out=boot_sb, in_=boot[:, :])
            qT_sb = const.tile([P, DT * PB * T], F16, name="qT_sb")
            nc.sync.dma_start(out=qT_sb[:, T:], in_=qT[:, T:])
            wq_sb = const.tile([P, DT * H], F16, name="wq_sb")
            nc.sync.dma_start(out=wq_sb[:, H:], in_=wq[:, H:])
            wrT_sb = const.tile([P, HT * D], F16, name="wrT_sb")
            nc.sync.dma_start(out=wrT_sb, in_=wrT[:, :])
            # mb2/bc are deferred into the rep-0 stream after the aT loads:
            # they are tiny but each DMA in the SP FIFO delays wrT/aT behind it
            mb2_sb = const.tile([2 * T, PB], F32, name="mb2_sb")
            bc_sb = const.tile([1, H], F16, name="bc_sb")
            wcy_sb = const.tile([P, HT * H], F16, name="wcy_sb")
            wca_sb = const.tile([P, DT * H], F16, name="wca_sb")
            wcp_sb = const.tile([P, HT * H], F16, name="wcp_sb")

            onef = const.tile([P, 1], F32, name="onef")
            nc.vector.memset(onef, 1.0)
            ones_col = const.tile([P, 1], BF16, name="ones_col")
            nc.vector.tensor_copy(ones_col[:], onef[:])
            ones_rf = const.tile([1, P], F32, name="ones_rf")
            nc.vector.memset(ones_rf, 1.0)
            ones_row_b = const.tile([1, P], BF16, name="ones_row_b")
            nc.vector.tensor_copy(ones_row_b[:], ones_rf[:])
            ones_row_h = const.tile([1, P], F16, name="ones_row_h")
            nc.vector.tensor_copy(ones_row_h[:], ones_rf[:])
            ident = const.tile([P, P], F32, name="ident")
            make_identity(nc, ident)
            ident_r = const.tile([P, P], F16, name="ident_r")
            nc.vector.tensor_copy(ident_r[:], ident[:])

            fillers = []

            def fill(n):
                for _ in range(min(n, len(fillers))):
                    fillers.pop(0)()

            def qphase_stages(i):
                """Projection pipeline for batch i as stage thunks.

                qn2/yn2 are [128, *] with the batch's [64, *] data duplicated
                in both partition halves (col-group-paired chains write both),
                so the chunk-packed attention stages can row/col-pair."""
                qn2 = perb.tile([P, H], F16, tag="qn", name="qn2")
                qt = perb.tile([P, HT * T], F16, tag="qt", name="qt")
                gT = perb.tile([T, D], F16, tag="gT", name="gT")
                g = perb.tile([P, DT * T], F16, tag="g", name="g")
                yn2 = perb.tile([P, H], F16, tag="yn", name="yn2")

                def s_qp():
                    ps_q = psum.tile([P, H], F32, tag="sc", bufs=2, name="ps_q")
                    for d in range(DT):
                        qsl = slice((d * PB + i) * T, (d * PB + i + 1) * T)
                        if d == 0:
                            lhsT = boot_sb[:, :T] if i == 0 else qT_sb[:, qsl]
                            rhs = boot_sb[:, T:]
                        else:
                            lhsT, rhs = qT_sb[:, qsl], wq_sb[:, d * H:(d + 1) * H]
                        for c in range(2):
                            nc.tensor.matmul(ps_q[c * T:(c + 1) * T, :], lhsT, rhs,
                                             start=(d == 0), stop=(d == DT - 1),
                                             skip_group_check=True)
                    nc.scalar.activation(qn2[:], ps_q[:], AFT.Copy)

                def s_qt():
                    for h in range(HT):
                        ps_qt = psum.tile([P, T], F16, tag="tr", bufs=2, name="ps_qt")
                        nc.tensor.transpose(ps_qt[:], qn2[:T, h * P:(h + 1) * P],
                                            ident_r[:T, :T])
                        nc.vector.tensor_copy(qt[:, h * T:(h + 1) * T], ps_qt[:])

                def s_g():
                    ps_g = psum.tile([T, D], F32, tag="sc", bufs=2, name="ps_g")
                    for h in range(HT):
                        nc.tensor.matmul(ps_g[:], qt[:, h * T:(h + 1) * T],
                                         wrT_sb[:, h * D:(h + 1) * D],
                                         start=(h == 0), stop=(h == HT - 1))
                    nc.scalar.activation(gT[:], ps_g[:], AFT.Copy)

                def s_gd():
                    for d in range(DT):
                        ps_gd = psum.tile([P, T], F16, tag="tr", bufs=2, name="ps_gd")
                        nc.tensor.transpose(ps_gd[:], gT[:, d * P:(d + 1) * P],
                                            ident_r[:T, :T])
                        nc.vector.tensor_copy(g[:, d * T:(d + 1) * T], ps_gd[:])

                def emit_y():
                    ps_y = psum.tile([P, H], F32, tag="sc", bufs=2, name="ps_y")
                    for h in range(HT):
                        for c in range(2):
                            nc.tensor.matmul(ps_y[c * T:(c + 1) * T, :],
                                             qt[:, h * T:(h + 1) * T],
                                             wcy_sb[:, h * H:(h + 1) * H],
                                             start=(h == 0), stop=False,
                                             skip_group_check=True)
                    for c in range(2):
                        nc.tensor.matmul(ps_y[c * T:(c + 1) * T, :],
                                         ones_row_h[:, :T], bc_sb[:],
                                         start=False, stop=True,
                                         skip_group_check=True)
                    nc.scalar.activation(yn2[:], ps_y[:], AFT.Copy)

                return qn2, qt, g, yn2, emit_y, [s_qp, s_qt, s_g, s_gd]

            def emit_qphase(i):
                qn2, qt, g, yn2, emit_y, stages = qphase_stages(i)
                for s in stages:
                    s()
                    fill(2)
                return qn2, qt, g, yn2, emit_y

            def attention(i, a0, a1, qn2, g, yn2, prefetch, first=False,
                          last=False):
                """Chunk-packed attention phase for batch i. Returns the
                prefetched aT tiles for batch i+1."""
                if first:
                    nc.sync.dma_start(out=wca_sb, in_=wc[:, 0:4 * H])
                if prefetch is not None:
                    t0 = chunk.tile([P, DT * ACH], F16, tag="aT", bufs=6, name="aT_t")
                    nc.sync.dma_start(out=t0, in_=aT[prefetch, 0, :, :])
                    t1 = chunk.tile([P, DT * ACH], F16, tag="aT", bufs=6, name="aT_t")
                    nc.sync.dma_start(out=t1, in_=aT[prefetch, 1, :, :])
                else:
                    t0 = t1 = None
                if first:
                    nc.sync.dma_start(out=wcp_sb, in_=wc[:, 8 * H:12 * H])

                # scores^T for both chunks: col-group-paired accumulation
                # chains (shared G weights, per-chunk anchors)
                ps_s = psum.tile([P, ACH], F32, tag="sc", bufs=2, name="ps_s")
                for d in range(DT):
                    for c, a in ((0, a0), (1, a1)):
                        nc.tensor.matmul(ps_s[c * T:(c + 1) * T, :],
                                         g[:, d * T:(d + 1) * T],
                                         a[:, d * ACH:(d + 1) * ACH],
                                         start=(d == 0), stop=(d == DT - 1),
                                         skip_group_check=True)
                attn_un = chunk.tile([P, ACH], BF16, tag="attn_un", name="attn_un")
                nc.scalar.activation(attn_un[:], ps_s[:], AFT.Exp,
                                     bias=mb2_sb[:, i:i + 1], scale=1.0)
                fill(3)
                # per-chunk Z column sums: two rank-64 matmuls, row-group
                # paired (contraction halves 0..63 / 64..127)
                ps_z0 = psum.tile([1, ACH], F32, tag="tr", bufs=2, name="ps_z0")
                ps_z1 = psum.tile([1, ACH], F32, tag="tr", bufs=2, name="ps_z1")
                nc.tensor.matmul(ps_z0[:], ones_col[:T, :], attn_un[:T, :],
                                 start=True, stop=True)
                nc.tensor.matmul(ps_z1[:], ones_col[T:2 * T, :],
                                 attn_un[T:2 * T, :], start=True, stop=True)
                rz0 = small.tile([1, ACH], BF16, tag="rz0", name="rz0")
                rz1 = small.tile([1, ACH], BF16, tag="rz1", name="rz1")
                with nc.allow_low_precision(reason="bf16 1/Z"):
                    nc.vector.reciprocal(rz0[:], ps_z0[:])
                    nc.vector.reciprocal(rz1[:], ps_z1[:])
                fill(3)
                # broadcast chunk c's 1/Z over its partition half (col-paired)
                ps_rz = psum.tile([P, ACH], F32, tag="tr", bufs=2, name="ps_rz")
                nc.tensor.matmul(ps_rz[:T, :], ones_row_b[:, :T], rz0[:],
                                 start=True, stop=True)
                nc.tensor.matmul(ps_rz[T:2 * T, :], ones_row_b[:, :T], rz1[:],
                                 start=True, stop=True)
                attn = chunk.tile([P, ACH], F16, tag="attn", name="attn")
                nc.vector.tensor_mul(attn[:], attn_un[:], ps_rz[:].bitcast(F32R))
                fill(3)
                # attended^T h-tiles, row-group-paired across chunks; attended
                # is only consumed via anchor*att, so multiply straight out of
                # PSUM
                pr_sb = [[None] * HT, [None] * HT]
                for h in range(HT):
                    ps_a0 = psum.tile([P, ACH], F32, tag="big", name="ps_a0")
                    nc.tensor.matmul(ps_a0[:], qn2[:T, h * P:(h + 1) * P],
                                     attn[:T, :], start=True, stop=True)
                    ps_a1 = psum.tile([P, ACH], F32, tag="big", name="ps_a1")
                    nc.tensor.matmul(ps_a1[:], qn2[T:2 * T, h * P:(h + 1) * P],
                                     attn[T:2 * T, :], start=True, stop=True)
                    for c, a, ps in ((0, a0, ps_a0), (1, a1, ps_a1)):
                        pr = chunk.tile([P, ACH], F16, tag=f"pr{c}{h}",
                                        name=f"pr{c}{h}")
                        nc.vector.tensor_mul(pr[:], a[:, h * ACH:(h + 1) * ACH],
                                             ps[:].bitcast(F32R))
                        pr_sb[c][h] = pr
                    fill(1)

                # deferred finals: x = relu([anchor|att|anchor*att] @ Wc + b)
                xo = [stage.tile([P, AT_CH * H], mybir.dt.bfloat16, tag="xo", name=f"xo{c}")
                      for c in range(2)]
                box = {}

                def groupA(c, a, j):
                    def emit():
                        ps_x = psum.tile([P, H], F32, tag="big", name="ps_x")
                        box[(c, j)] = ps_x
                        for d in range(DT):
                            nc.tensor.matmul(
                                ps_x[:], a[:, d * ACH + j * P:d * ACH + (j + 1) * P],
                                wca_sb[:, d * H:(d + 1) * H],
                                start=(d == 0), stop=False)
                    return emit

                def groupY(j):
                    def emit():
                        jsl = slice(j * P, (j + 1) * P)
                        nc.tensor.matmul(box[(0, j)][:], attn[:T, jsl],
                                         yn2[:T, :], start=False, stop=False)
                        nc.tensor.matmul(box[(1, j)][:], attn[T:2 * T, jsl],
                                         yn2[T:2 * T, :], start=False, stop=False)
                    return emit

                def groupB(c, j):
                    def emit():
                        jsl = slice(j * P, (j + 1) * P)
                        ps_x = box.pop((c, j))
                        for h in range(HT):
                            nc.tensor.matmul(ps_x[:], pr_sb[c][h][:, jsl],
                                             wcp_sb[:, h * H:(h + 1) * H],
                                             start=False, stop=(h == HT - 1))
                        nc.scalar.activation(xo[c][:, j * H:(j + 1) * H], ps_x[:],
                                             AFT.Relu)
                        if last:
                            # tail: per-j DMA so the output stream drains
                            # while the remaining finals still compute
                            nc.scalar.dma_start(out=x[i, c, :, j * H:(j + 1) * H],
                                                in_=xo[c][:, j * H:(j + 1) * H])
                        elif j == AT_CH // 2 - 1:
                            nc.scalar.dma_start(out=x[i, c, :, :AT_CH * H // 2],
                                                in_=xo[c][:, :AT_CH * H // 2])
                        elif j == AT_CH - 1:
                            nc.scalar.dma_start(out=x[i, c, :, AT_CH * H // 2:],
                                                in_=xo[c][:, AT_CH * H // 2:])
                    return emit

                for j in range(AT_CH):
                    fillers.append(groupA(0, a0, j))
                    fillers.append(groupA(1, a1, j))
                    fillers.append(groupY(j))
                    fillers.append(groupB(0, j))
                    fillers.append(groupB(1, j))
                return t0, t1

            for rep in range(reps):
                a0 = chunk.tile([P, DT * ACH], F16, tag="aT", bufs=6, name="aT_t")
                nc.sync.dma_start(out=a0, in_=aT[0, 0, :, :])
                a1 = chunk.tile([P, DT * ACH], F16, tag="aT", bufs=6, name="aT_t")
                nc.sync.dma_start(out=a1, in_=aT[0, 1, :, :])
                if rep == 0:
                    nc.sync.dma_start(out=mb2_sb, in_=mb2[:, :])
                    nc.sync.dma_start(out=bc_sb, in_=bc[:, :])
                    nc.sync.dma_start(out=wcy_sb, in_=wc[:, 4 * H:8 * H])
                # batches 0 and 1 projected together: their interleaved
                # stages cover each other's cross-engine latency
                q0 = qphase_stages(0)
                q1 = qphase_stages(1)
                for s0, s1 in zip(q0[5], q1[5]):
                    s0()
                    s1()
                    fill(3)
                qph = {0: q0, 1: q1}
                if rep > 0:
                    q0[4]()
                    q1[4]()
                for i in range(PB):
                    qn2, qt, g, yn2, emit_y = qph[i][:5]
                    nxt = i + 1 if i + 1 < PB else None
                    first = (i == 0 and rep == 0)
                    last = (i == PB - 1 and rep == reps - 1)
                    t0, t1 = attention(i, a0, a1, qn2, g, yn2,
                                       prefetch=nxt, first=first, last=last)
                    if rep == 0 and i == 0:
                        qph[0][4]()   # deferred Y (wcy just landed)
                        qph[1][4]()
                    if nxt is not None and nxt >= 2:
                        qph[nxt] = emit_qphase(nxt)
                        qph[nxt][4]()
                        fill(1)
                    a0, a1 = t0, t1
            while fillers:
                fillers.pop(0)()
    nc.compile()
    return nc


def _prep(anchor_feats, query_embs, query_mask, W_region, W_query, W_combine, b_combine):
    """Host-side shard + layout prep. Returns the 8 per-core input maps."""
    f = np.float32
    hf = np.float16 if FP16 else np.float32
    NC = NCORES
    a = np.asarray(anchor_feats, dtype=f)
    aT = np.ascontiguousarray(
        a.reshape(NC, PB, NCH, ACH, DT, P).transpose(0, 1, 2, 5, 4, 3)
    ).reshape(NC, PB, NCH, P, DT * ACH).astype(hf)
    q = np.asarray(query_embs, dtype=f)
    qT = np.ascontiguousarray(
        q.reshape(NC, PB, T, DT, P).transpose(0, 4, 3, 1, 2)
    ).reshape(NC, P, DT * PB * T).astype(hf)
    m = np.asarray(query_mask).reshape(NC, PB, T)
    mb = np.ascontiguousarray(
        np.where(m > 0, f(-SHIFT), f(-1e9)).transpose(0, 2, 1))
    mb2 = np.concatenate([mb, mb], axis=1)  # [NC, 2T, PB]
    wq = np.ascontiguousarray(
        np.asarray(W_query, dtype=f).reshape(DT, P, H).transpose(1, 0, 2)
    ).reshape(P, DT * H).astype(hf)
    wrT = np.ascontiguousarray(
        np.asarray(W_region, dtype=f).T.reshape(HT, P, D).transpose(1, 0, 2)
    ).reshape(P, HT * D).astype(hf)
    wcv = np.ascontiguousarray(
        np.asarray(W_combine, dtype=f).reshape(CT, P, H).transpose(1, 0, 2)
    ).reshape(P, CT * H).astype(hf)
    bcv = np.ascontiguousarray(np.asarray(b_combine, dtype=f)).reshape(1, H).astype(hf)
    return [
        {"boot": np.ascontiguousarray(
            np.concatenate([qT[cid][:, :T], wq[:, :H]], axis=1)),
         "aT": aT[cid], "qT": qT[cid], "mb2": mb2[cid],
         "wq": wq, "wrT": wrT, "wc": wcv, "bc": bcv}
        for cid in range(NC)
    ]


def kernel(anchor_feats, query_embs, query_mask,
           W_region, W_query, W_combine, b_combine):
    if "nc" not in _CACHE:
        _CACHE["nc"] = build()
    nc = _CACHE["nc"]
    in_maps = _prep(anchor_feats, query_embs, query_mask,
                    W_region, W_query, W_combine, b_combine)
    res = run_bass_kernel_spmd(nc, in_maps, core_ids=list(range(NCORES)))
    out = np.empty((B, A, H), dtype=np.float32)
    for cid in range(NCORES):
        xd = np.asarray(res.results[cid]["x"], dtype=np.float32)
        xd = xd.reshape(PB, NCH, P, AT_CH, H).transpose(0, 1, 3, 2, 4)
        out[cid * PB:(cid + 1) * PB] = xd.reshape(PB, A, H)
    return out


# revision 28
# speedup vs baseline: 1.3513x; 1.3513x over previous
"""CrossModalAttentionScorer Trainium2 kernel, v11 (Bass/Tile, 8 NeuronCores).

Same math as v3 (G-trick for scores, attn@Y for the combine's attended
block) plus two structural changes:

  1. fp16 everywhere on the score/combine path (11-bit mantissa ~= f32r
     precision for these ~N(0,1) values, half the DMA bytes); bf16 only for
     the unnormalized softmax path (exp(s-60) spans up to e^30, beyond fp16
     range). All matmuls are same-dtype (the BIR verifier forbids mixing
     f32/f32r with 16-bit operands).

  2. Both 512-row chunks of a batch are processed together, packed along
     the 128-partition axis (chunk 0 in partitions 0..63, chunk 1 in
     64..127). Every T=64-tall stage then runs as two concurrent 64-wide
     PE-array tiles via tile_position (auto-derived from base partitions):
       - scores: 2 col-groups (one per chunk, shared G weights)
       - attended / attn@Y: 2 row-groups (disjoint contraction halves)
       - exp / Z / 1/Z / normalize: single full-128-partition instructions
     This halves the PE time of every half-width stage; on HW the paired
     matmuls overlap almost perfectly (the cost model charges them serially,
     so TimelineSim over-reports this kernel).

The combine matmuls stay deferred (fillers) and are woven into the next
batch's q-phase / attention-phase latency gaps; leftovers drain only after
the LAST rep so they cover the next rep's boundary windows. G^T tiles are
computed directly as 16 small N=64 matmuls (no [T,D] matmul + PE-transpose
round-trip). Output DMAs go per-j on the final batch so the tail pipelines;
the startup DMA FIFO is ordered strictly by first use.
"""
import numpy as np

import concourse.bacc as bacc
import concourse.tile as tile
import concourse.mybir as mybir
from concourse.bass_utils import run_bass_kernel_spmd
from concourse.masks import make_identity

B, A, T, D, H = 32, 1024, 64, 512, 512
NCORES = 8
PB = B // NCORES          # batches per core = 4
P = 128                   # partitions
DT = D // P               # 4 d-tiles
HT = H // P               # 4 h-tiles
CT = 3 * H // P           # 12 c-tiles (concat dim)
ACH = 512                 # a-chunk (moving-dim) size
NCH = A // ACH            # 2 chunks per batch (packed along partitions)
AT_CH = ACH // P          # 4 a-tiles per chunk
SHIFT = 60.0              # fixed softmax shift (replaces row max)

F32 = mybir.dt.float32
F32R = mybir.dt.float32r
AFT = mybir.ActivationFunctionType

# FP16 mode: score/combine path in fp16 (half DMA bytes, ~f32r precision),
# softmax unnormalized path in bf16 (range). FP16=False reverts every matmul
# operand to f32r (v3 dtypes) while keeping the v5 packed structure.
FP16 = True
F16 = mybir.dt.float16 if FP16 else mybir.dt.float32r
BF16 = mybir.dt.bfloat16 if FP16 else mybir.dt.float32r

_CACHE = {}


def build(reps: int = 1):
    """Build the per-core Bass module (4 batches of the problem).

    reps>1 repeats the whole computation in one NEFF for slope timing."""
    nc = bacc.Bacc(None, target_bir_lowering=False, debug=False)

    aT = nc.dram_tensor("aT", [PB, NCH, P, DT * ACH], F16, kind="ExternalInput")
    boot = nc.dram_tensor("boot", [P, T + H], F16, kind="ExternalInput")
    qT = nc.dram_tensor("qT", [P, DT * PB * T], F16, kind="ExternalInput")
    mb2 = nc.dram_tensor("mb2", [2 * T, PB], F32, kind="ExternalInput")
    wq = nc.dram_tensor("wq", [P, DT * H], F16, kind="ExternalInput")
    wrT = nc.dram_tensor("wrT", [P, HT * D], F16, kind="ExternalInput")
    wc = nc.dram_tensor("wc", [P, CT * H], F16, kind="ExternalInput")
    bc = nc.dram_tensor("bc", [1, H], F16, kind="ExternalInput")
    x = nc.dram_tensor("x", [PB, NCH, P, AT_CH * H], mybir.dt.bfloat16, kind="ExternalOutput")

    with tile.TileContext(nc) as tc:
        with (
            tc.tile_pool(name="const", bufs=1) as const,
            tc.tile_pool(name="perb", bufs=3) as perb,
            tc.tile_pool(name="chunk", bufs=2) as chunk,
            tc.tile_pool(name="small", bufs=4) as small,
            tc.tile_pool(name="stage", bufs=4) as stage,
            tc.tile_pool(name="psum", bufs=4, space="PSUM") as psum,
        ):
            # ---- constants (load order = need order) ----
            boot_sb = const.tile([P, T + H], F16, name="boot_sb")
            nc.sync.dma_start(out=boot_sb, in_=boot[:, :])
            qT_sb = const.tile([P, DT * PB * T], F16, name="qT_sb")
            nc.sync.dma_start(out=qT_sb[:, T:], in_=qT[:, T:])
            wq_sb = const.tile([P, DT * H], F16, name="wq_sb")
            nc.sync.dma_start(out=wq_sb[:, H:], in_=wq[:, H:])
            wrT_sb = const.tile([P, HT * D], F16, name="wrT_sb")
            nc.sync.dma_start(out=wrT_sb, in_=wrT[:, :])
            # mb2/bc are deferred into the rep-0 stream after the aT loads:
            # they are tiny but each DMA in the SP FIFO delays wrT/aT behind it
            mb2_sb = const.tile([2 * T, PB], F32, name="mb2_sb")
            bc_sb = const.tile([1, H], F16, name="bc_sb")
            wcy_sb = const.tile([P, HT * H], F16, name="wcy_sb")
            wca_sb = const.tile([P, DT * H], F16, name="wca_sb")
            wcp_sb = const.tile([P, HT * H], F16, name="wcp_sb")

            onef = const.tile([P, 1], F32, name="onef")
            nc.vector.memset(onef, 1.0)
            ones_col = const.tile([P, 1], BF16, name="ones_col")
            nc.vector.tensor_copy(ones_col[:], onef[:])
            ones_rf = const.tile([1, P], F32, name="ones_rf")
            nc.vector.memset(ones_rf, 1.0)
            ones_row_b = const.tile([1, P], BF16, name="ones_row_b")
            nc.vector.tensor_copy(ones_row_b[:], ones_rf[:])
            ones_row_h = const.tile([1, P], F16, name="ones_row_h")
            nc.vector.tensor_copy(ones_row_h[:], ones_rf[:])
            ident = const.tile([P, P], F32, name="ident")
            make_identity(nc, ident)
            ident_r = const.tile([P, P], F16, name="ident_r")
            nc.vector.tensor_copy(ident_r[:], ident[:])

            fillers = []

            def fill(n):
                for _ in range(min(n, len(fillers))):
                    fillers.pop(0)()

            def qphase_stages(i):
                """Projection pipeline for batch i as stage thunks.

                qn2/yn2 are [128, *] with the batch's [64, *] data duplicated
                in both partition halves (col-group-paired chains write both),
                so the chunk-packed attention stages can row/col-pair."""
                qn2 = perb.tile([P, H], F16, tag="qn", name="qn2")
                qt = perb.tile([P, HT * T], F16, tag="qt", name="qt")
                g = perb.tile([P, DT * T], F16, tag="g", name="g")
                yn2 = perb.tile([P, H], F16, tag="yn", name="yn2")

                def s_qp():
                    ps_q = psum.tile([P, H], F32, tag="sc", bufs=2, name="ps_q")
                    for d in range(DT):
                        qsl = slice((d * PB + i) * T, (d * PB + i + 1) * T)
                        if d == 0:
                            lhsT = boot_sb[:, :T] if i == 0 else qT_sb[:, qsl]
                            rhs = boot_sb[:, T:]
                        else:
                            lhsT, rhs = qT_sb[:, qsl], wq_sb[:, d * H:(d + 1) * H]
                        for c in range(2):
                            nc.tensor.matmul(ps_q[c * T:(c + 1) * T, :], lhsT, rhs,
                                             start=(d == 0), stop=(d == DT - 1),
                                             skip_group_check=True)
                    nc.scalar.activation(qn2[:], ps_q[:], AFT.Copy)

                def s_qt():
                    for h in range(HT):
                        ps_qt = psum.tile([P, T], F16, tag="tr", bufs=2, name="ps_qt")
                        nc.tensor.transpose(ps_qt[:], qn2[:T, h * P:(h + 1) * P],
                                            ident_r[:T, :T])
                        nc.vector.tensor_copy(qt[:, h * T:(h + 1) * T], ps_qt[:])

                def s_g():
                    # direct G^T tiles: g[:, dT:(d+1)T] = sum_h wrT(h,d).T @ QpT(h)
                    # (16 small N=64 matmuls; replaces the [T,D] matmul + ACT
                    # copy + 4 PE transposes, and rounds to fp16 only once)
                    for d in range(DT):
                        ps_gd = psum.tile([P, T], F32, tag="tr", bufs=2, name="ps_gd")
                        for h in range(HT):
                            nc.tensor.matmul(
                                ps_gd[:],
                                wrT_sb[:, h * D + d * P:h * D + (d + 1) * P],
                                qt[:, h * T:(h + 1) * T],
                                start=(h == 0), stop=(h == HT - 1))
                        nc.vector.tensor_copy(g[:, d * T:(d + 1) * T], ps_gd[:])

                def emit_y():
                    ps_y = psum.tile([P, H], F32, tag="sc", bufs=2, name="ps_y")
                    for h in range(HT):
                        for c in range(2):
                            nc.tensor.matmul(ps_y[c * T:(c + 1) * T, :],
                                             qt[:, h * T:(h + 1) * T],
                                             wcy_sb[:, h * H:(h + 1) * H],
                                             start=(h == 0), stop=False,
                                             skip_group_check=True)
                    for c in range(2):
                        nc.tensor.matmul(ps_y[c * T:(c + 1) * T, :],
                                         ones_row_h[:, :T], bc_sb[:],
                                         start=False, stop=True,
                                         skip_group_check=True)
                    nc.scalar.activation(yn2[:], ps_y[:], AFT.Copy)

                return qn2, qt, g, yn2, emit_y, [s_qp, s_qt, s_g]

            def emit_qphase(i):
                qn2, qt, g, yn2, emit_y, stages = qphase_stages(i)
                for s in stages:
                    s()
                    fill(3)
                return qn2, qt, g, yn2, emit_y

            def attention(i, a0, a1, qn2, g, yn2, prefetch, first=False,
                          last=False):
                """Chunk-packed attention phase for batch i. Returns the
                prefetched aT tiles for batch i+1."""
                if first:
                    nc.sync.dma_start(out=wca_sb, in_=wc[:, 0:4 * H])
                if prefetch is not None:
                    t0 = chunk.tile([P, DT * ACH], F16, tag="aT", bufs=6, name="aT_t")
                    nc.sync.dma_start(out=t0, in_=aT[prefetch, 0, :, :])
                    t1 = chunk.tile([P, DT * ACH], F16, tag="aT", bufs=6, name="aT_t")
                    nc.sync.dma_start(out=t1, in_=aT[prefetch, 1, :, :])
                else:
                    t0 = t1 = None
                if first:
                    nc.sync.dma_start(out=wcp_sb, in_=wc[:, 8 * H:12 * H])

                # scores^T for both chunks: col-group-paired accumulation
                # chains (shared G weights, per-chunk anchors)
                ps_s = psum.tile([P, ACH], F32, tag="sc", bufs=2, name="ps_s")
                for d in range(DT):
                    for c, a in ((0, a0), (1, a1)):
                        nc.tensor.matmul(ps_s[c * T:(c + 1) * T, :],
                                         g[:, d * T:(d + 1) * T],
                                         a[:, d * ACH:(d + 1) * ACH],
                                         start=(d == 0), stop=(d == DT - 1),
                                         skip_group_check=True)
                attn_un = chunk.tile([P, ACH], BF16, tag="attn_un", name="attn_un")
                nc.scalar.activation(attn_un[:], ps_s[:], AFT.Exp,
                                     bias=mb2_sb[:, i:i + 1], scale=1.0)
                fill(3)
                # per-chunk Z column sums: two rank-64 matmuls, row-group
                # paired (contraction halves 0..63 / 64..127)
                ps_z0 = psum.tile([1, ACH], F32, tag="tr", bufs=2, name="ps_z0")
                ps_z1 = psum.tile([1, ACH], F32, tag="tr", bufs=2, name="ps_z1")
                nc.tensor.matmul(ps_z0[:], ones_col[:T, :], attn_un[:T, :],
                                 start=True, stop=True)
                nc.tensor.matmul(ps_z1[:], ones_col[T:2 * T, :],
                                 attn_un[T:2 * T, :], start=True, stop=True)
                rz0 = small.tile([1, ACH], BF16, tag="rz0", name="rz0")
                rz1 = small.tile([1, ACH], BF16, tag="rz1", name="rz1")
                with nc.allow_low_precision(reason="bf16 1/Z"):
                    nc.vector.reciprocal(rz0[:], ps_z0[:])
                    nc.vector.reciprocal(rz1[:], ps_z1[:])
                fill(3)
                # broadcast chunk c's 1/Z over its partition half (col-paired)
                ps_rz = psum.tile([P, ACH], F32, tag="tr", bufs=2, name="ps_rz")
                nc.tensor.matmul(ps_rz[:T, :], ones_row_b[:, :T], rz0[:],
                                 start=True, stop=True)
                nc.tensor.matmul(ps_rz[T:2 * T, :], ones_row_b[:, :T], rz1[:],
                                 start=True, stop=True)
                attn = chunk.tile([P, ACH], F16, tag="attn", name="attn")
                nc.vector.tensor_mul(attn[:], attn_un[:], ps_rz[:].bitcast(F32R))
                fill(3)
                # attended^T h-tiles, row-group-paired across chunks; attended
                # is only consumed via anchor*att, so multiply straight out of
                # PSUM
                pr_sb = [[None] * HT, [None] * HT]
                for h in range(HT):
                    ps_a0 = psum.tile([P, ACH], F32, tag="big", name="ps_a0")
                    nc.tensor.matmul(ps_a0[:], qn2[:T, h * P:(h + 1) * P],
                                     attn[:T, :], start=True, stop=True)
                    ps_a1 = psum.tile([P, ACH], F32, tag="big", name="ps_a1")
                    nc.tensor.matmul(ps_a1[:], qn2[T:2 * T, h * P:(h + 1) * P],
                                     attn[T:2 * T, :], start=True, stop=True)
                    for c, a, ps in ((0, a0, ps_a0), (1, a1, ps_a1)):
                        pr = chunk.tile([P, ACH], F16, tag=f"pr{c}{h}",
                                        name=f"pr{c}{h}")
                        nc.vector.tensor_mul(pr[:], a[:, h * ACH:(h + 1) * ACH],
                                             ps[:].bitcast(F32R))
                        pr_sb[c][h] = pr
                    fill(1)

                # deferred finals: x = relu([anchor|att|anchor*att] @ Wc + b)
                xo = [stage.tile([P, AT_CH * H], mybir.dt.bfloat16, tag="xo", name=f"xo{c}")
                      for c in range(2)]
                box = {}

                def groupA(c, a, j):
                    def emit():
                        ps_x = psum.tile([P, H], F32, tag="big", name="ps_x")
                        box[(c, j)] = ps_x
                        for d in range(DT):
                            nc.tensor.matmul(
                                ps_x[:], a[:, d * ACH + j * P:d * ACH + (j + 1) * P],
                                wca_sb[:, d * H:(d + 1) * H],
                                start=(d == 0), stop=False)
                    return emit

                def groupY(j):
                    def emit():
                        jsl = slice(j * P, (j + 1) * P)
                        nc.tensor.matmul(box[(0, j)][:], attn[:T, jsl],
                                         yn2[:T, :], start=False, stop=False)
                        nc.tensor.matmul(box[(1, j)][:], attn[T:2 * T, jsl],
                                         yn2[T:2 * T, :], start=False, stop=False)
                    return emit

                def groupB(c, j):
                    def emit():
                        jsl = slice(j * P, (j + 1) * P)
                        ps_x = box.pop((c, j))
                        for h in range(HT):
                            nc.tensor.matmul(ps_x[:], pr_sb[c][h][:, jsl],
                                             wcp_sb[:, h * H:(h + 1) * H],
                                             start=False, stop=(h == HT - 1))
                        nc.scalar.activation(xo[c][:, j * H:(j + 1) * H], ps_x[:],
                                             AFT.Relu)
                        if last:
                            # tail: per-j DMA so the output stream drains
                            # while the remaining finals still compute
                            nc.scalar.dma_start(out=x[i, c, :, j * H:(j + 1) * H],
                                                in_=xo[c][:, j * H:(j + 1) * H])
                        elif j == AT_CH // 2 - 1:
                            nc.scalar.dma_start(out=x[i, c, :, :AT_CH * H // 2],
                                                in_=xo[c][:, :AT_CH * H // 2])
                        elif j == AT_CH - 1:
                            nc.scalar.dma_start(out=x[i, c, :, AT_CH * H // 2:],
                                                in_=xo[c][:, AT_CH * H // 2:])
                    return emit

                for j in range(AT_CH):
                    fillers.append(groupA(0, a0, j))
                    fillers.append(groupA(1, a1, j))
                    fillers.append(groupY(j))
                    fillers.append(groupB(0, j))
                    fillers.append(groupB(1, j))
                return t0, t1

            for rep in range(reps):
                a0 = chunk.tile([P, DT * ACH], F16, tag="aT", bufs=6, name="aT_t")
                nc.sync.dma_start(out=a0, in_=aT[0, 0, :, :])
                a1 = chunk.tile([P, DT * ACH], F16, tag="aT", bufs=6, name="aT_t")
                nc.sync.dma_start(out=a1, in_=aT[0, 1, :, :])
                if rep == 0:
                    nc.sync.dma_start(out=mb2_sb, in_=mb2[:, :])
                    nc.sync.dma_start(out=bc_sb, in_=bc[:, :])
                    nc.sync.dma_start(out=wcy_sb, in_=wc[:, 4 * H:8 * H])
                # batches 0 and 1 projected together: their interleaved
                # stages cover each other's cross-engine latency
                q0 = qphase_stages(0)
                q1 = qphase_stages(1)
                for s0, s1 in zip(q0[5], q1[5]):
                    s0()
                    s1()
                    fill(3)
                qph = {0: q0, 1: q1}
                if rep > 0:
                    q0[4]()
                    q1[4]()
                for i in range(PB):
                    qn2, qt, g, yn2, emit_y = qph[i][:5]
                    nxt = i + 1 if i + 1 < PB else None
                    first = (i == 0 and rep == 0)
                    last = (i == PB - 1 and rep == reps - 1)
                    t0, t1 = attention(i, a0, a1, qn2, g, yn2,
                                       prefetch=nxt, first=first, last=last)
                    if rep == 0 and i == 0:
                        qph[0][4]()   # deferred Y (wcy just landed)
                        qph[1][4]()
                    if nxt is not None and nxt >= 2:
                        qph[nxt] = emit_qphase(nxt)
                        qph[nxt][4]()
                        fill(1)
                    a0, a1 = t0, t1
            while fillers:
                fillers.pop(0)()
    nc.compile()
    return nc


def _prep(anchor_feats, query_embs, query_mask, W_region, W_query, W_combine, b_combine):
    """Host-side shard + layout prep. Returns the 8 per-core input maps."""
    f = np.float32
    hf = np.float16 if FP16 else np.float32
    NC = NCORES
    a = np.asarray(anchor_feats, dtype=f)
    aT = np.ascontiguousarray(
        a.reshape(NC, PB, NCH, ACH, DT, P).transpose(0, 1, 2, 5, 4, 3)
    ).reshape(NC, PB, NCH, P, DT * ACH).astype(hf)
    q = np.asarray(query_embs, dtype=f)
    qT = np.ascontiguousarray(
        q.reshape(NC, PB, T, DT, P).transpose(0, 4, 3, 1, 2)
    ).reshape(NC, P, DT * PB * T).astype(hf)
    m = np.asarray(query_mask).reshape(NC, PB, T)
    mb = np.ascontiguousarray(
        np.where(m > 0, f(-SHIFT), f(-1e9)).transpose(0, 2, 1))
    mb2 = np.concatenate([mb, mb], axis=1)  # [NC, 2T, PB]
    wq = np.ascontiguousarray(
        np.asarray(W_query, dtype=f).reshape(DT, P, H).transpose(1, 0, 2)
    ).reshape(P, DT * H).astype(hf)
    wrT = np.ascontiguousarray(
        np.asarray(W_region, dtype=f).T.reshape(HT, P, D).transpose(1, 0, 2)
    ).reshape(P, HT * D).astype(hf)
    wcv = np.ascontiguousarray(
        np.asarray(W_combine, dtype=f).reshape(CT, P, H).transpose(1, 0, 2)
    ).reshape(P, CT * H).astype(hf)
    bcv = np.ascontiguousarray(np.asarray(b_combine, dtype=f)).reshape(1, H).astype(hf)
    return [
        {"boot": np.ascontiguousarray(
            np.concatenate([qT[cid][:, :T], wq[:, :H]], axis=1)),
         "aT": aT[cid], "qT": qT[cid], "mb2": mb2[cid],
         "wq": wq, "wrT": wrT, "wc": wcv, "bc": bcv}
        for cid in range(NC)
    ]


def kernel(anchor_feats, query_embs, query_mask,
           W_region, W_query, W_combine, b_combine):
    if "nc" not in _CACHE:
        _CACHE["nc"] = build()
    nc = _CACHE["nc"]
    in_maps = _prep(anchor_feats, query_embs, query_mask,
                    W_region, W_query, W_combine, b_combine)
    res = run_bass_kernel_spmd(nc, in_maps, core_ids=list(range(NCORES)))
    out = np.empty((B, A, H), dtype=np.float32)
    for cid in range(NCORES):
        xd = np.asarray(res.results[cid]["x"], dtype=np.float32)
        xd = xd.reshape(PB, NCH, P, AT_CH, H).transpose(0, 1, 3, 2, 4)
        out[cid * PB:(cid + 1) * PB] = xd.reshape(PB, A, H)
    return out
